# revision 11
# baseline (speedup 1.0000x reference)
"""Query-axis-softmax attention on 8 trn2 cores — v2.

Math (per head): scores = q @ k.T / sqrt(64); masked entries -> -1e9;
attn = softmax(scores, axis=QUERY); out = attn @ v.

Layout trick (unchanged from v1): scores kept TRANSPOSED as [k, q], so the
softmax reduction (over q) runs along the free axis and the PV product is a
plain matmul contracting on the partition axis.

v2 changes vs v1 (cost-model + HW-slope driven):
 - the mask is applied as a {0,1} fp16 gate multiply on the vector engine
   AFTER the exp (exp(s+m) == exp(s)*gate), removing the per-head identity
   matmuls from the PE entirely (was 1/3 of PE work).
 - scalar-engine Exp no longer carries accum_out (the accumulator read-out
   serializes the activation pipeline on HW, +~1us/instr); row sums come
   from a DVE tensor_scalar pass (4x perf mode) over the [128, 2048] p tile.
 - PV operands are swapped (p stationary, vsc moving): output lands in the
   natural [q, d] layout, each head accumulates in its own PSUM banks (no
   bank write-port sharing between the packed heads), and the moving
   operand shrinks from 512 to 64 rows per matmul.
 - PV matmuls are software-pipelined 2 strips behind the score matmuls,
   and each pair's PSUM->SBUF eviction rides inside the next pair's
   stream, so the scalar engine never starves at strip/pair boundaries.
 - mask DMAs ride the Pool engine queue, off the q/k/v critical path.

Sharding: 32 (b*h) heads -> 4 per core, no cross-core communication.
"""

import numpy as np
import ml_dtypes

B, H, S, DK = 2, 16, 2048, 64
N_CORES = 8
HPC = (B * H) // N_CORES  # heads per core
P = 128                   # sbuf partitions
NSTRIP = S // P           # 16 strips of k-rows
HF = 1024                 # exp half-strip width (2 PSUM banks)
MASK_BIAS = -240.0        # exact in fp8e4; exp(0.125*(score-240)) < 4e-11

_CACHE = {}

# Defaults chosen by TimelineSim + HW-slope search (see transcript):
# DVE mask gate, swapped PV operands (p stationary, per-head PSUM banks),
# 2-strip PV lag, PV emitted 2 qk-groups into the strip.
DEFAULT_OPTS = dict(mask_mode="dve", pv_lag=2, p_bufs=6, emit_pos=2,
                    pv_swap=True, pv_split=True, gate_half=True)


def _build(reps=1, mask_mode="dr", act_accum=False, hi_exp=10, sc_bufs=2,
           out_copy_pool=False, body_mult=1, pv_lag=1, emit_pos=0,
           p_bufs=4, no_pv=False, no_exp=False, no_gate=False,
           no_sums=False, pv_const_w=False, pv_swap=False,
           fuse_gate=False, gate_scratch=False, pv_split=True,
           pv_stride=1, vsc_pool=False, gate_half=False, p_fp8=False):
    import contextlib

    import concourse.tile as tile
    from concourse import mybir, bacc

    f32 = mybir.dt.float32
    f16 = mybir.dt.float16
    fp8 = mybir.dt.float8e4
    pdt = fp8 if p_fp8 else f16
    Exp = mybir.ActivationFunctionType.Exp
    Alu = mybir.AluOpType
    DR = mybir.MatmulPerfMode.DoubleRow

    nc = bacc.Bacc(None, target_bir_lowering=False)
    qT = nc.dram_tensor("qT", [HPC, DK, S], f16, kind="ExternalInput")
    kT = nc.dram_tensor("kT", [HPC, DK, S], f16, kind="ExternalInput")
    v = nc.dram_tensor("v", [HPC, S, DK], f32, kind="ExternalInput")
    if mask_mode == "dr":
        mT = nc.dram_tensor("mT", [NSTRIP, 64, 2, S], fp8, kind="ExternalInput")
        idDR = nc.dram_tensor("idDR", [64, 2, P], fp8, kind="ExternalInput")
    elif mask_mode == "dve":
        # {0,1} gate; fp16 gets the DVE 2x perf mode, fp8 halves SBUF traffic
        mT = nc.dram_tensor("mT", [S, S], pdt, kind="ExternalInput")
    else:
        mT = nc.dram_tensor("mT", [S, S], fp8, kind="ExternalInput")
    out_shape = [HPC, S, DK] if pv_swap else [HPC, DK, S]
    outT = nc.dram_tensor("outT", out_shape, f32, kind="ExternalOutput")

    with tile.TileContext(nc) as tc:
        with (
            tc.tile_pool(name="mask", bufs=1) as mask_pool,
            tc.tile_pool(name="const", bufs=1) as const_pool,
            tc.tile_pool(name="qk", bufs=2) as qk_pool,
            tc.tile_pool(name="vload", bufs=2) as v_pool,
            tc.tile_pool(name="p", bufs=p_bufs) as p_pool,
            tc.tile_pool(name="small", bufs=8) as small_pool,
            tc.tile_pool(name="outsb", bufs=2) as out_pool,
            tc.tile_pool(name="scps", bufs=sc_bufs, space="PSUM") as sc_psum,
            tc.tile_pool(name="outps", bufs=1, space="PSUM") as out_psum_pool,
        ):
            bias_ap = None
            if p_fp8:
                bias_t = const_pool.tile([P, 1], f32, name="bias_t")
                nc.vector.memset(bias_t[:], -2.0)
                bias_ap = bias_t
            cw = None
            if pv_const_w:
                cw = const_pool.tile([P, DK], f16, name="cw")
                nc.vector.memset(cw[:], 0.01)
            ident = None
            if mask_mode == "dr":
                ident = const_pool.tile([64, 2, P], fp8)
                nc.sync.dma_start(ident[:], idDR[:, :, :])
            elif mask_mode == "plain":
                from concourse.masks import make_identity

                ident = const_pool.tile([P, P], fp8)
                make_identity(nc, ident[:])

            # Whole mask stays resident in SBUF.  Loaded via the Pool
            # engine's DGE queue so the mask traffic doesn't sit in front of
            # the first pair's q/k/v loads on the SP queue.
            mask_tiles = []
            for s in range(NSTRIP):
                if mask_mode == "dr":
                    mt = mask_pool.tile([64, 2, S], fp8, tag=f"m{s}")
                    nc.gpsimd.dma_start(mt[:], mT[s])
                else:
                    mdt = pdt if mask_mode == "dve" else fp8
                    mt = mask_pool.tile([P, S], mdt, tag=f"m{s}")
                    nc.gpsimd.dma_start(mt[:], mT[s * P:(s + 1) * P, :])
                mask_tiles.append(mt)

            def qk_mask_exp(kview, qview, mstrip, s, hf, qh, pt, accum):
                """Scores for one [128k, HF q] block of one head (rows half
                `hf` of the packed pair), then exp into pt."""
                sc = sc_psum.tile([P, HF], f32)
                for sub in range(HF // 512):
                    cols = slice(sub * 512, (sub + 1) * 512)
                    q0 = sub * 512
                    qg = qh * HF + q0  # global q offset into the mask strip
                    nc.tensor.matmul(
                        sc[:, cols],
                        lhsT=kview[:, s * P:(s + 1) * P],
                        rhs=qview[:, q0:q0 + 512],
                        start=True,
                        stop=(mask_mode == "dve"),
                        tile_position=(64 * hf, 0),
                    )
                    if mask_mode == "dr":
                        nc.tensor.matmul(
                            sc[:, cols],
                            lhsT=ident[:],
                            rhs=mstrip[:, :, qg:qg + 512],
                            start=False,
                            stop=True,
                            perf_mode=DR,
                        )
                    elif mask_mode == "plain":
                        nc.tensor.matmul(
                            sc[:, cols],
                            lhsT=ident[:],
                            rhs=mstrip[:, qg:qg + 512],
                            start=False,
                            stop=True,
                        )
                prio = (
                    tc.high_priority(hi_exp) if hi_exp
                    else contextlib.nullcontext()
                )
                if no_exp:
                    if not (no_pv and no_gate and no_sums):
                        nc.vector.memset(pt, 1.0)
                else:
                    with prio:
                        nc.scalar.activation(
                            out=pt, in_=sc[:], func=Exp, scale=0.125,
                            # fp8 p: shift the exp down so a 6-sigma score
                            # can't overflow fp8e4's 448 max; the constant
                            # cancels in the softmax normalization.
                            bias=bias_ap[:] if p_fp8 else 0.0,
                            accum_out=accum if act_accum else None,
                        )

            def make_emit_pv_split(out_ps, s, pts, vscs, tail):
                """Swap-mode PV as 4 sub-closures (hf x chunk-half) so the
                in-order PE queue interleaves PV work finely between QK
                groups and the scalar engine never starves."""
                def chunk_batch(hf, c0, c1, is_last):
                    def go():
                        if not no_pv:
                            for c in range(c0, c1, pv_stride):
                                nc.tensor.matmul(
                                    out_ps[hf][:, c * DK:(c + 1) * DK],
                                    lhsT=pts[hf][:, c * P:(c + 1) * P],
                                    rhs=cw[:] if pv_const_w else vscs[hf][:],
                                    start=False,
                                    stop=(s == NSTRIP - 1),
                                    skip_group_check=True,
                                )
                        if is_last and tail is not None:
                            tail()

                    return go

                h = NSTRIP // 2
                return [
                    chunk_batch(0, 0, h, False),
                    chunk_batch(0, h, NSTRIP, False),
                    chunk_batch(1, 0, h, False),
                    chunk_batch(1, h, NSTRIP, True),
                ]

            def make_emit_pv(out_ps, s, pts, vscs, tail):
                def go():
                    for hf in range(2 if not no_pv else 0):
                        if pv_swap:
                            # p stationary / vsc moving: out[q,d] natural,
                            # each head in its own PSUM banks, 64-row moving.
                            # The 16 chunk groups share PSUM zero regions, so
                            # the tile is pre-zeroed by memset and every
                            # matmul accumulates (start=False).
                            for c in range(0, NSTRIP, pv_stride):
                                nc.tensor.matmul(
                                    out_ps[hf][:, c * DK:(c + 1) * DK],
                                    lhsT=pts[hf][:, c * P:(c + 1) * P],
                                    rhs=cw[:] if pv_const_w else vscs[hf][:],
                                    start=False,
                                    stop=(s == NSTRIP - 1),
                                    skip_group_check=True,
                                )
                            continue
                        for qc in range(4):
                            cols = slice(qc * 512, (qc + 1) * 512)
                            nc.tensor.matmul(
                                out_ps[64 * hf:64 * (hf + 1), cols],
                                lhsT=cw[:] if pv_const_w else vscs[hf][:],
                                rhs=pts[hf][:, cols],
                                start=(s == 0),
                                stop=(s == NSTRIP - 1),
                                tile_position=(0, 64 * hf),
                                # A/B col-tiles share the bank but write
                                # disjoint partition halves; the sim's group
                                # check is address-only and would reject it.
                                skip_group_check=True,
                            )
                    if tail is not None:
                        tail()

                return go

            def make_tail(out_ps, hA, hB):
                def go():
                    if pv_swap:
                        for hf, h in ((0, hA), (1, hB)):
                            osb = out_pool.tile(
                                [P, NSTRIP, DK], f32, tag=f"osb{hf}",
                                name="osb"
                            )
                            if no_pv:
                                nc.vector.memset(osb[:], 0.0)
                            else:
                                nc.vector.tensor_copy(osb[:], out_ps[hf][:])
                            nc.gpsimd.dma_start(
                                outT[h].rearrange("(c p) d -> p c d", p=P),
                                osb[:],
                            )
                        return
                    out_sb = out_pool.tile([P, S], f32, name="out_sb")
                    if no_pv:
                        nc.vector.memset(out_sb[:], 0.0)
                    elif out_copy_pool:
                        nc.gpsimd.tensor_copy(out_sb[:], out_ps[:])
                    else:
                        nc.vector.tensor_copy(out_sb[:], out_ps[:])
                    nc.gpsimd.dma_start(outT[hA], out_sb[0:DK, :])
                    nc.gpsimd.dma_start(outT[hB], out_sb[DK:P, :])

                return go

            # Software pipeline across pairs: strip s's PV matmuls are
            # emitted `pv_lag` strips later, `emit_pos` qk-groups into a
            # strip, so the scalar engine gets fresh sc tiles before PE
            # turns to PV work and the sums->recip->vsc chain stays off the
            # critical path.  Each pair's PSUM->SBUF out copy + store rides
            # behind its final PV group, inside the next pair's stream.
            pending = []
            qk_ctr = 0
            loop_cm = (
                tc.For_i(0, reps, 1) if reps > 1 else contextlib.nullcontext()
            )
            with loop_cm:
              for hp in [x for _ in range(body_mult) for x in range(HPC // 2)]:
                hA, hB = 2 * hp, 2 * hp + 1
                qts = qk_pool.tile([P, S], f16, tag="q")
                kts = qk_pool.tile([P, S], f16, tag="k")
                qtviews = [qts[0:DK, :], qts[DK:P, :]]
                ktviews = [kts[0:DK, :], kts[DK:P, :]]
                vts = v_pool.tile([P, 2, NSTRIP, DK], f32, tag="v")
                nc.sync.dma_start(qtviews[0], qT[hA])
                nc.sync.dma_start(qtviews[1], qT[hB])
                nc.sync.dma_start(ktviews[0], kT[hA])
                nc.sync.dma_start(ktviews[1], kT[hB])
                nc.sync.dma_start(
                    vts[:, 0], v[hA].rearrange("(s p) d -> p s d", p=P)
                )
                nc.sync.dma_start(
                    vts[:, 1], v[hB].rearrange("(s p) d -> p s d", p=P)
                )

                if pv_swap:
                    out_ps = [
                        out_psum_pool.tile(
                            [P, NSTRIP * DK], f32, tag=f"o{hf}", name="out_ps"
                        )
                        for hf in range(2)
                    ]
                    for hf in range(2):
                        nc.vector.memset(out_ps[hf][:], 0.0)
                else:
                    out_ps = out_psum_pool.tile([P, S], f32, name="out_ps")

                for s in range(NSTRIP):
                    mstrip = mask_tiles[s]
                    pts = []
                    vscs = []
                    for hf in range(2):  # head A=0 / head B=1 of the pair
                        pt = p_pool.tile([P, S], pdt, tag=f"p{hf}")
                        stot = small_pool.tile(
                            [P, 1], f32, tag=f"stot{hf}", name="stot"
                        )
                        if gate_half:
                            hsum = small_pool.tile(
                                [P, 2], f32, tag=f"hsum{hf}", name="hsum"
                            )
                        if act_accum:
                            ssum = small_pool.tile(
                                [P, 2], f32, tag=f"ssum{hf}", name="ssum"
                            )
                        for qh in range(2):  # q halves
                            qk_mask_exp(
                                ktviews[hf],
                                qtviews[hf][:, qh * HF:(qh + 1) * HF],
                                mstrip,
                                s,
                                hf,
                                qh,
                                pt[:, qh * HF:(qh + 1) * HF],
                                ssum[:, qh:qh + 1] if act_accum else None,
                            )
                            if gate_half and mask_mode == "dve":
                                hcols = slice(qh * HF, (qh + 1) * HF)
                                if not no_gate:
                                    nc.vector.tensor_tensor(
                                        out=pt[:, hcols], in0=pt[:, hcols],
                                        in1=mstrip[:, hcols], op=Alu.mult,
                                    )
                                if not no_sums:
                                    nc.vector.tensor_scalar(
                                        out=pt[:, hcols], in0=pt[:, hcols],
                                        scalar1=1.0, scalar2=None,
                                        op0=Alu.mult, op1=Alu.add,
                                        accum_out=hsum[:, qh:qh + 1],
                                    )
                            if pv_swap and pv_split:
                                if len(pending) > 4 * (pv_lag - 1):
                                    pending.pop(0)()
                            elif (
                                len(pending) > (pv_lag - 1)
                                and qk_ctr % 4 == emit_pos
                            ):
                                pending.pop(0)()
                            qk_ctr += 1
                        if gate_half and mask_mode == "dve":
                            if no_sums:
                                nc.vector.memset(stot[:], 1.0)
                            else:
                                nc.vector.tensor_add(
                                    stot[:], hsum[:, 0:1], hsum[:, 1:2]
                                )
                        elif mask_mode == "dve" and not no_gate and fuse_gate:
                            # fused gate multiply + row-sum reduce: one DVE
                            # instruction per strip-head
                            nc.vector.tensor_tensor_reduce(
                                out=pt[:], in0=pt[:], in1=mstrip[:],
                                scale=1.0, scalar=0.0,
                                op0=Alu.mult, op1=Alu.add,
                                accum_out=stot[:],
                            )
                        elif mask_mode == "dve" and not no_gate:
                            # apply the {0,1} mask gate post-exp, then row
                            # sums at 4x perf mode
                            if gate_scratch:
                                ptg = p_pool.tile(
                                    [P, S], f16, tag=f"pg{hf}", name="ptg"
                                )
                                nc.vector.tensor_tensor(
                                    out=ptg[:], in0=pt[:], in1=mstrip[:],
                                    op=Alu.mult,
                                )
                                pt = ptg
                            else:
                                nc.vector.tensor_tensor(
                                    out=pt[:], in0=pt[:], in1=mstrip[:],
                                    op=Alu.mult,
                                )
                        if gate_half and mask_mode == "dve":
                            if no_sums:
                                nc.vector.memset(stot[:], 1.0)
                            else:
                                nc.vector.tensor_add(
                                    stot[:], hsum[:, 0:1], hsum[:, 1:2]
                                )
                        elif mask_mode == "dve" and not no_gate and fuse_gate:
                            pass  # sums came from the fused reduce above
                        elif act_accum:
                            nc.vector.tensor_add(
                                stot[:], ssum[:, 0:1], ssum[:, 1:2]
                            )
                        elif no_sums:
                            nc.vector.memset(stot[:], 1.0)
                        else:
                            # row sums via DVE 4x-mode pass (in-place copy)
                            nc.vector.tensor_scalar(
                                out=pt[:], in0=pt[:], scalar1=1.0,
                                scalar2=None, op0=Alu.mult, op1=Alu.add,
                                accum_out=stot[:],
                            )
                        # vsc eagerly, off the PV critical path
                        sinv = small_pool.tile(
                            [P, 1], f32, tag=f"sinv{hf}", name="sinv"
                        )
                        nc.vector.reciprocal(sinv[:], stot[:])
                        vsc = small_pool.tile(
                            [P, DK], f16, tag=f"vsc{hf}", name="vsc"
                        )
                        if vsc_pool:
                            nc.gpsimd.tensor_scalar_mul(
                                vsc[:], vts[:, hf, s, :], sinv[:]
                            )
                        else:
                            nc.vector.tensor_scalar_mul(
                                vsc[:], vts[:, hf, s, :], sinv[:]
                            )
                        pts.append(pt)
                        vscs.append(vsc)
                    tail = (
                        make_tail(out_ps, hA, hB)
                        if s == NSTRIP - 1 else None
                    )
                    if pv_swap and pv_split:
                        pending.extend(
                            make_emit_pv_split(out_ps, s, pts, vscs, tail)
                        )
                    else:
                        pending.append(
                            make_emit_pv(out_ps, s, pts, vscs, tail)
                        )
              for item in pending:
                item()
              pending = []

    nc.compile()
    return nc


def get_nc(**opts):
    key = tuple(sorted(opts.items()))
    if key not in _CACHE:
        _CACHE[key] = _build(**opts)
    return _CACHE[key]


def make_in_maps(q, k, v, mask, mask_mode="dr", p_fp8=False):
    """Full inputs -> list of 8 per-core input maps."""
    q32 = np.asarray(q, np.float32).reshape(B * H, S, DK)
    k32 = np.asarray(k, np.float32).reshape(B * H, S, DK)
    v32 = np.ascontiguousarray(np.asarray(v, np.float32).reshape(B * H, S, DK))
    qT = np.ascontiguousarray(q32.transpose(0, 2, 1)).astype(np.float16)
    kT = np.ascontiguousarray(k32.transpose(0, 2, 1)).astype(np.float16)
    maskT = np.asarray(mask).reshape(S, S).T            # [k, q]
    mdt = ml_dtypes.float8_e4m3
    common = {}
    if mask_mode == "dr":
        mTb = np.where(maskT, np.float32(MASK_BIAS), np.float32(0.0)).astype(mdt)
        # strip s, partition p, half i holds mask row 128*s + 64*i + p
        common["mT"] = np.ascontiguousarray(
            mTb.reshape(NSTRIP, 2, 64, S).transpose(0, 2, 1, 3)
        )
        ident = np.zeros((64, 2, P), mdt)
        for p_ in range(64):
            ident[p_, 0, p_] = 1.0
            ident[p_, 1, p_ + 64] = 1.0
        common["idDR"] = ident
    elif mask_mode == "dve":
        gdt = ml_dtypes.float8_e4m3 if p_fp8 else np.float16
        common["mT"] = np.where(maskT, 0.0, 1.0).astype(gdt)
    else:
        common["mT"] = np.where(
            maskT, np.float32(MASK_BIAS), np.float32(0.0)
        ).astype(mdt)
    in_maps = []
    for c in range(N_CORES):
        sl = slice(c * HPC, (c + 1) * HPC)
        in_maps.append(
            {
                "qT": np.ascontiguousarray(qT[sl]),
                "kT": np.ascontiguousarray(kT[sl]),
                "v": v32[sl],
                **common,
            }
        )
    return in_maps


def assemble_out(per_core_outT):
    """8 x [HPC, DK, S] (or [HPC, S, DK] for pv_swap) -> [B, H, S, DK]."""
    out = np.concatenate([np.asarray(o) for o in per_core_outT], axis=0)
    if out.shape[1] == DK:  # transposed layout [DK, S]
        out = out.reshape(B, H, DK, S).transpose(0, 1, 3, 2)
    else:
        out = out.reshape(B, H, S, DK)
    return np.ascontiguousarray(out).astype(np.float32)


def kernel(q, k, v, mask):
    from concourse.bass_utils import run_bass_kernel_spmd

    nc = get_nc(**DEFAULT_OPTS)
    in_maps = make_in_maps(
        q, k, v, mask, mask_mode=DEFAULT_OPTS["mask_mode"],
        p_fp8=DEFAULT_OPTS.get("p_fp8", False),
    )
    res = run_bass_kernel_spmd(nc, in_maps, core_ids=list(range(N_CORES)))
    return assemble_out([r["outT"] for r in res.results])
